# revision 66
# baseline (speedup 1.0000x reference)
"""Trainium2 Bass kernel for nn_Luong_61684320305412 (bidirectional masked
softmax attention, B=8, L0=L1=2048, D=256).

Sharding: data-parallel over batch B across the 8 NeuronCores (one batch
element per core). Per core:

    S   = q0 @ q1^T + NEG * m0[:,None]*m1[None,:]
    E   = exp(S/256)                 (masked entries underflow to exactly 0)
    out0 = (E @ q1)    / rowsum(E) / 16
    out1 = (E^T @ q0)  / colsum(E) / 16

Implementation (fp8 DoubleRow design):
  - All big matmuls use fp8e4m3 inputs with perf_mode=DoubleRow, which packs
    the K=256 contraction into a single PE pass (2 fp8 weights per cell).
  - The mask outer product is a rank-1 K=1 fp8 matmul (+-224 encodings;
    (-224*224)/256 = -196 -> exp underflows to 0 exactly). K=1 matmuls are
    row-tiled via tile_position so up to 4 run concurrently in the PE array.
  - E is stored centered: e = E - 1 in fp8 (values in [-1, 0.45]), which cuts
    fp8 quantization noise ~12x where it matters. The identity part of
    E = 1 + e is restored algebraically:
        out0^T = q1_8^T @ e  (+ c1A (x) (1-m0) + c1B (x) m0)  [rank-2 f32r MM]
    where c1A = sum_m q1[m,:] (exact f32) and c1B uses the quantized q1 on
    masked columns so the e = -1 cancellation is exact.
  - Out-matmuls run "swapped" (values stationary, e moving) producing out^T
    in PSUM with d on partitions; per-partition c-rows are added by the
    rank-2 matmul, tiles are evicted to bf16, PE-transposed back, and
    normalized by the reciprocal row/col sums (captured for free via the
    exp activation's accum_out).
  - Host-side prep (numpy): fp8 casts, transposed copies, mask/c rows. This
    is layout/sharding work on ~4 MB/core and keeps the device kernel lean.
"""

from contextlib import ExitStack

import numpy as np
import ml_dtypes

import concourse.bass as bass
import concourse.tile as tile
from concourse import bacc, mybir
from concourse.bass_utils import run_bass_kernel_spmd
from concourse.masks import make_identity

P = 128
B = 8
L = 2048          # L0 == L1
D = 256
T = L // P        # 16 row tiles
NCH = 512         # psum bank width in fp32
MC = 224.0        # mask encoding; (-224*224)/256 = -196 -> exp -> exactly 0
SCALE2 = 1.0 / 256.0   # applied to scores inside exp
SCALE1 = 1.0 / 16.0    # applied to the averaged values at the end

f32 = mybir.dt.float32
f32r = mybir.dt.float32r
bf16 = mybir.dt.bfloat16
f8 = mybir.dt.float8e4
MUL = mybir.AluOpType.mult
EXP = mybir.ActivationFunctionType.Exp
DR = mybir.MatmulPerfMode.DoubleRow
DRSW = mybir.MatmulPerfMode.DoubleRowSwInterleave

F8NP = ml_dtypes.float8_e4m3fn


def _emit(tc: tile.TileContext, ctx: ExitStack, io: dict):
    nc = tc.nc

    consts = ctx.enter_context(tc.tile_pool(name="consts", bufs=1))
    qpool = ctx.enter_context(tc.tile_pool(name="qpool", bufs=1))
    epool = ctx.enter_context(tc.tile_pool(name="epool", bufs=1))
    ebf = ctx.enter_context(tc.tile_pool(name="ebf", bufs=4))
    posb_pool = ctx.enter_context(tc.tile_pool(name="posb", bufs=4))
    outsb = ctx.enter_context(tc.tile_pool(name="outsb", bufs=4))

    # ---- input layouts (dram images are already partition-major: straight
    # block copies, chunked so the first score tiles can start early) ----
    q0n = qpool.tile([P, T, D], f8)       # q0 fp8, row l = t*128+p
    q1n = qpool.tile([P, T, D], f8)
    q0t = qpool.tile([P, 2, L], f8)       # q0^T fp8, d = ko*128+ki
    q1t = qpool.tile([P, 2, L], f8)
    # the A-phase rhs (q1t, needed in full) loads first; the lhsT (q0t) is
    # consumed per 128-column tile, so it streams in behind on a third ring
    s0 = io["q0t"].rearrange("p (ko l) -> p ko l", ko=2)
    s1 = io["q1t"].rearrange("p (ko l) -> p ko l", ko=2)
    nc.sync.dma_start(out=q1t[:, 0, :], in_=s1[:, 0, :])
    nc.sync.dma_start(out=q1t[:, 1, :], in_=s1[:, 1, :])
    nc.sync.dma_start(out=q0t[:, :, 0:256], in_=s0[:, :, 0:256])

    # ---- mask rows (fp8, +-224), replicated at partitions 0/32/64/96 for
    # row-tiled K=1 matmuls; dim1: 0 = -224*m0, 1 = +224*m1 ----
    mtile = consts.tile([128, 2, L], f8)
    for g in range(4):
        nc.sync.dma_start(out=mtile[g * 32 : g * 32 + 1, :, :], in_=io["mrows"][g : g + 1, :, :])
    nc.sync.dma_start(out=q0t[:, :, 256:1024], in_=s0[:, :, 256:1024])
    nc.sync.dma_start(out=q0t[:, :, 1024:2048], in_=s0[:, :, 1024:2048])

    # q natural layouts are first needed by the out1 chains in phase B
    nc.sync.dma_start(out=q0n, in_=io["q0n"].rearrange("p (t d) -> p t d", t=T))
    nc.sync.dma_start(out=q1n, in_=io["q1n"].rearrange("p (t d) -> p t d", t=T))

    # ---- rank-2 correction operands (f32r), pairs at partitions (32g, 32g+1),
    # packed in one dram tensor (8 row DMAs; needed only ~40us in):
    # ctile dim1: 0 = (c1A, c1B) for out0, 1 = (c0A, c0B) for out1
    # rtile dim1: 0 = (1-m0, m0),          1 = (1-m1, m1) ----
    crm = consts.tile([128, 2, D + L], f32r)
    ctile = crm[:, :, 0:D]
    rtile = crm[:, :, D : D + L]
    for g in range(4):
        nc.sync.dma_start(
            out=crm[g * 32 : g * 32 + 2, :, :],
            in_=io["crm"][2 * g : 2 * g + 2, :, :].bitcast(f32r),
        )

    identf = consts.tile([P, P], f32)
    make_identity(nc, identf)
    identb = consts.tile([P, P], bf16)
    nc.vector.tensor_copy(out=identb, in_=identf)

    # ---- e tiles (fp8, centered E-1) and row/col sums ----
    eA = epool.tile([P, T, L], f8)        # [l0-part(t), m]   feeds out1
    eB = epool.tile([P, T, L], f8)        # [l1-part(t), l0]  feeds out0
    denA = consts.tile([P, T], f32)       # rowsum(E) per tile
    denB = consts.tile([P, 2, T], f32)    # per-half partials in phase B
    d16A = consts.tile([P, T], f32)
    d16B = consts.tile([P, T], f32)
    rc0 = consts.tile([P, T], f32)        # (1/16)/rowsum(E)  (out0 denominators)
    rc1 = consts.tile([P, T], f32)        # (1/16)/colsum(E)  (out1 denominators)

    def score_tile_a(s_psum, t):
        ps = s_psum.tile([P, L], f32, tag="sp")
        for c in range(4):
            off = c * NCH
            nc.tensor.matmul(
                ps[:, off : off + NCH],
                lhsT=q0t[:, 0:2, t * P : (t + 1) * P],
                rhs=q1t[:, 0:2, off : off + NCH],
                start=True,
                stop=False,
                perf_mode=DR,
            )
        for c in range(4):
            off = c * NCH
            nc.tensor.matmul(
                ps[:, off : off + NCH],
                lhsT=mtile[c * 32 : c * 32 + 1, 0, t * P : (t + 1) * P],
                rhs=mtile[c * 32 : c * 32 + 1, 1, off : off + NCH],
                start=False,
                stop=True,
                tile_position=(c * 32, 0),
            )
        eb = ebf.tile([P, L], bf16, tag="ebf")
        nc.scalar.activation(
            out=eb, in_=ps, func=EXP, scale=SCALE2, accum_out=denA[:, t : t + 1]
        )
        nc.vector.tensor_scalar_add(out=eA[:, t, :], in0=eb, scalar1=-1.0)

    def score_tile_b(s_psum, t):
        # both half tiles' DR matmuls first, then all four K=1 mask matmuls
        # back-to-back so they row-tile 4x concurrently in the PE array
        pss = []
        for h in range(2):
            ps = s_psum.tile([P, 2 * NCH], f32, tag="sp", name=f"psb{h}")
            pss.append(ps)
            for c in range(2):
                off = h * 1024 + c * NCH
                nc.tensor.matmul(
                    ps[:, c * NCH : (c + 1) * NCH],
                    lhsT=q1t[:, 0:2, t * P : (t + 1) * P],
                    rhs=q0t[:, 0:2, off : off + NCH],
                    start=True,
                    stop=False,
                    perf_mode=DR,
                )
        for h in range(2):
            for c in range(2):
                off = h * 1024 + c * NCH
                g = h * 2 + c
                nc.tensor.matmul(
                    pss[h][:, c * NCH : (c + 1) * NCH],
                    lhsT=mtile[g * 32 : g * 32 + 1, 1, t * P : (t + 1) * P],
                    rhs=mtile[g * 32 : g * 32 + 1, 0, off : off + NCH],
                    start=False,
                    stop=True,
                    tile_position=(g * 32, 0),
                )
        for h in range(2):
            eb = ebf.tile([P, 2 * NCH], bf16, tag="ebf")
            nc.scalar.activation(
                out=eb, in_=pss[h], func=EXP, scale=SCALE2,
                accum_out=denB[:, h, t : t + 1],
            )
            nc.vector.tensor_scalar_add(
                out=eB[:, t, h * 1024 : (h + 1) * 1024], in0=eb, scalar1=-1.0
            )
        if True:
            nc.vector.tensor_tensor(
                out=d16B[:, t : t + 1],
                in0=denB[:, 0, t : t + 1],
                in1=denB[:, 1, t : t + 1],
                op=mybir.AluOpType.add,
            )
            nc.vector.tensor_scalar_mul(
                out=d16B[:, t : t + 1], in0=d16B[:, t : t + 1], scalar1=16.0
            )
            nc.vector.reciprocal(rc1[:, t : t + 1], d16B[:, t : t + 1])

    def out_chain(o_psum, wq, ev, jj, mg, nch, posb):
        po = o_psum.tile([P, NCH], f32, tag="op")
        for k in range(8):
            nc.tensor.matmul(
                po,
                lhsT=wq[:, 2 * k + mg, :],
                rhs=ev[:, 2 * k : 2 * k + 2, nch * NCH : (nch + 1) * NCH],
                start=(k == 0),
                stop=False,
                perf_mode=DRSW,
            )
        nc.tensor.matmul(
            po,
            lhsT=ctile[nch * 32 : nch * 32 + 2, jj, mg * P : (mg + 1) * P],
            rhs=rtile[nch * 32 : nch * 32 + 2, jj, nch * NCH : (nch + 1) * NCH],
            start=False,
            stop=True,
            tile_position=(nch * 32, 0),
        )
        nc.scalar.copy(out=posb[mg][:, nch * NCH : (nch + 1) * NCH], in_=po)

    def finish_pair(t_psum, posb, rc, odram, t):
        # two adjacent row tiles: 4 transposes -> one psum buf -> 2 scaled
        # copies -> a single batched store
        tp = t_psum.tile([P, 2, D], bf16, tag="tp")
        for u in range(2):
            for mg in range(2):
                nc.tensor.transpose(
                    tp[:, u, mg * P : (mg + 1) * P],
                    posb[mg][:, (t + u) * P : (t + u + 1) * P],
                    identb,
                )
        ob = outsb.tile([P, 2, D], f32, tag="ob")
        for u in range(2):
            nc.scalar.activation(
                out=ob[:, u, :], in_=tp[:, u, :],
                func=mybir.ActivationFunctionType.Copy,
                scale=rc[:, t + u : t + u + 1],
            )
        nc.sync.dma_start(
            out=odram[t * P : (t + 2) * P, :].rearrange("(u p) d -> p u d", p=P),
            in_=ob,
        )

    posb1 = [
        posb_pool.tile([P, L], bf16, tag=f"po1{mg}", name=f"posb1_{mg}")
        for mg in range(2)
    ]
    posb0 = [
        posb_pool.tile([P, L], bf16, tag=f"po0{mg}", name=f"posb0_{mg}")
        for mg in range(2)
    ]

    # ---- phase A: orientation A scores (S[l0, m] -> eA, rowsum -> rc0) ----
    with tc.tile_pool(name="s_psumA", bufs=2, space="PSUM") as s_psumA:
        for t in range(T):
            score_tile_a(s_psumA, t)
    # rc0 = 1/(16*rowsum(E)), batched (rc0 is only read by the out0 finishes)
    nc.vector.tensor_scalar_mul(out=d16A, in0=denA, scalar1=16.0)
    nc.vector.reciprocal(rc0, d16A)

    # ---- phase B: orientation B scores interleaved with out1 work (out1
    # depends only on eA, so its matmuls fill the PE behind the B exps).
    # out1's 8 accumulation chains (9 MMs each) are dribbled out ~5 MMs per
    # score tile; once both mg-chunks of a 512-column group are evicted, its
    # finish tiles (transpose + normalize + store) ride along as well. ----
    o_psum = ctx.enter_context(tc.tile_pool(name="o_psum", bufs=2, space="PSUM"))
    t_psum = ctx.enter_context(tc.tile_pool(name="t_psum", bufs=2, space="PSUM"))
    with tc.tile_pool(name="s_psumB", bufs=2, space="PSUM") as s_psumB:
        mm_cursor = [0]  # flat index over out1's 8 chains x 9 matmuls
        po_cur = [None]
        rc1_ready = [-1]          # highest tile whose rc1 has been emitted
        pending_fin = []          # nch groups whose finishes await rc1

        def flush_finishes():
            rest = []
            for nch in pending_fin:
                if 4 * nch + 3 <= rc1_ready[0]:
                    for tt in range(nch * 4, nch * 4 + 4, 2):
                        finish_pair(t_psum, posb1, rc1, io["out1"], tt)
                else:
                    rest.append(nch)
            pending_fin[:] = rest

        def emit_out1_mms(n):
            for _ in range(n):
                idx = mm_cursor[0]
                if idx >= 72:
                    return
                j, k = divmod(idx, 9)
                mg, nch = j % 2, j // 2
                if k == 0:
                    po_cur[0] = o_psum.tile([P, NCH], f32, tag="op", name="po1c")
                po = po_cur[0]
                if k < 8:
                    nc.tensor.matmul(
                        po,
                        lhsT=q0n[:, 2 * k + mg, :],
                        rhs=eA[:, 2 * k : 2 * k + 2, nch * NCH : (nch + 1) * NCH],
                        start=(k == 0),
                        stop=False,
                        perf_mode=DRSW,
                    )
                else:
                    nc.tensor.matmul(
                        po,
                        lhsT=ctile[nch * 32 : nch * 32 + 2, 1, mg * P : (mg + 1) * P],
                        rhs=rtile[nch * 32 : nch * 32 + 2, 1, nch * NCH : (nch + 1) * NCH],
                        start=False,
                        stop=True,
                        tile_position=(nch * 32, 0),
                    )
                    nc.scalar.copy(
                        out=posb1[mg][:, nch * NCH : (nch + 1) * NCH], in_=po
                    )
                    if mg == 1:
                        pending_fin.append(nch)
                        flush_finishes()
                mm_cursor[0] += 1

        for t in range(T):
            score_tile_b(s_psumB, t)
            rc1_ready[0] = t
            if t >= 1:
                emit_out1_mms(5 if t < 15 else 72)
                flush_finishes()
        emit_out1_mms(72)
        flush_finishes()
        assert not pending_fin and mm_cursor[0] >= 72

        # ---- out0 chains with finish pairs staggered one pair behind ----
        next_pair = [0]

        def flush_out0_pairs(upto):
            while next_pair[0] <= upto:
                finish_pair(t_psum, posb0, rc0, io["out0"], next_pair[0] * 2)
                next_pair[0] += 1

        for nch in range(4):
            for mg in range(2):
                out_chain(o_psum, q1n, eB, 0, mg, nch, posb0)
            flush_out0_pairs(2 * nch - 1)  # lag one group behind ready
        flush_out0_pairs(7)


_CACHED_NC = None


def _build():
    global _CACHED_NC
    if _CACHED_NC is not None:
        return _CACHED_NC
    nc = bacc.Bacc("TRN2", target_bir_lowering=False, debug=False)
    io = {
        "q0n": nc.dram_tensor("q0n", [P, T * D], f8, kind="ExternalInput").ap(),
        "q1n": nc.dram_tensor("q1n", [P, T * D], f8, kind="ExternalInput").ap(),
        "q0t": nc.dram_tensor("q0t", [P, 2 * L], f8, kind="ExternalInput").ap(),
        "q1t": nc.dram_tensor("q1t", [P, 2 * L], f8, kind="ExternalInput").ap(),
        "mrows": nc.dram_tensor("mrows", [4, 2, L], f8, kind="ExternalInput").ap(),
        "crm": nc.dram_tensor("crm", [8, 2, D + L], f32, kind="ExternalInput").ap(),
        "out0": nc.dram_tensor("out0", [L, D], f32, kind="ExternalOutput").ap(),
        "out1": nc.dram_tensor("out1", [L, D], f32, kind="ExternalOutput").ap(),
    }
    with tile.TileContext(nc) as tc:
        with ExitStack() as ctx:
            _emit(tc, ctx, io)
    nc.compile()
    _CACHED_NC = nc
    return nc


def _prep_inputs(q0, q1, m0, m1):
    """Host-side sharding/layout prep for one batch element (numpy)."""
    q0_8 = q0.astype(F8NP)
    q1_8 = q1.astype(F8NP)
    q0_8f = q0_8.astype(np.float32)
    q1_8f = q1_8.astype(np.float32)
    m0f = m0.astype(np.float32)
    m1f = m1.astype(np.float32)

    mrows = np.empty([4, 2, L], F8NP)
    mrows[:, 0, :] = (-MC * m0f).astype(F8NP)[None, :]
    mrows[:, 1, :] = (MC * m1f).astype(F8NP)[None, :]

    # c-rows: identity-part restoration. A-row: all columns at full precision;
    # B-row: masked columns use the quantized values so the e=-1 cancellation
    # in masked rows is exact.
    c1A = q1.sum(0)
    c1B = np.where(m1f[:, None] == 1.0, q1_8f, q1).sum(0)
    c0A = q0.sum(0)
    c0B = np.where(m0f[:, None] == 1.0, q0_8f, q0).sum(0)
    crm = np.empty([8, 2, D + L], np.float32)
    for g in range(4):
        crm[2 * g, 0, :D] = c1A
        crm[2 * g + 1, 0, :D] = c1B
        crm[2 * g, 1, :D] = c0A
        crm[2 * g + 1, 1, :D] = c0B
        crm[2 * g, 0, D:] = 1.0 - m0f
        crm[2 * g + 1, 0, D:] = m0f
        crm[2 * g, 1, D:] = 1.0 - m1f
        crm[2 * g + 1, 1, D:] = m1f

    # partition-major dram images: [p, (t, d)] for the natural layout and
    # [ki, (ko, l)] for the transposed layout -> straight [128, X] DMAs
    def nat_img(q8a):
        # sw-interleaved out-chain weight windows: j = 2k+mg -> [128, 256]
        # (pairs from m-tiles 2k/2k+1, d-columns mg*128..+128, interleaved
        # with columns reversed as the DoubleRowSwInterleave path expects)
        qn = q8a.reshape(T, P, D).transpose(1, 0, 2)  # [ki, t, d]
        win = np.empty([P, T, 2 * P], q8a.dtype)
        for k in range(8):
            for mg in range(2):
                a = qn[:, 2 * k, mg * P : (mg + 1) * P]
                b = qn[:, 2 * k + 1, mg * P : (mg + 1) * P]
                win[:, 2 * k + mg, 0::2] = a[:, ::-1]
                win[:, 2 * k + mg, 1::2] = b[:, ::-1]
        return np.ascontiguousarray(win.reshape(P, T * D))

    def t_img(q8a):
        return np.ascontiguousarray(
            q8a.T.reshape(2, P, L).transpose(1, 0, 2).reshape(P, 2 * L)
        )

    return {
        "q0n": nat_img(q0_8),
        "q1n": nat_img(q1_8),
        "q0t": t_img(q0_8),
        "q1t": t_img(q1_8),
        "mrows": mrows,
        "crm": crm,
    }


def run_on_cores(q0, q1, mask0, mask1, trace=False):
    """Run the SPMD kernel; returns (out0, out1, BassKernelResults)."""
    nc = _build()
    in_maps = [
        _prep_inputs(
            np.asarray(q0[b], dtype=np.float32),
            np.asarray(q1[b], dtype=np.float32),
            np.asarray(mask0[b], dtype=np.int32),
            np.asarray(mask1[b], dtype=np.int32),
        )
        for b in range(B)
    ]
    br = run_bass_kernel_spmd(nc, in_maps, list(range(B)), trace=trace)
    out0 = np.stack([br.results[b]["out0"] for b in range(B)])
    out1 = np.stack([br.results[b]["out1"] for b in range(B)])
    return out0, out1, br


def kernel(q0, q1, len0=None, len1=None, mask0=None, mask1=None, **_):
    q0 = np.asarray(q0, dtype=np.float32)
    q1 = np.asarray(q1, dtype=np.float32)
    mask0 = np.asarray(mask0, dtype=np.int32)
    mask1 = np.asarray(mask1, dtype=np.int32)
    out0, out1, _br = run_on_cores(q0, q1, mask0, mask1, trace=False)
    return out0, out1


# revision 67
# speedup vs baseline: 1.0122x; 1.0122x over previous
"""Trainium2 Bass kernel for nn_Luong_61684320305412 (bidirectional masked
softmax attention, B=8, L0=L1=2048, D=256).

Sharding: data-parallel over batch B across the 8 NeuronCores (one batch
element per core). Per core:

    S   = q0 @ q1^T + NEG * m0[:,None]*m1[None,:]
    E   = exp(S/256)                 (masked entries underflow to exactly 0)
    out0 = (E @ q1)    / rowsum(E) / 16
    out1 = (E^T @ q0)  / colsum(E) / 16

Implementation (fp8 DoubleRow design):
  - All big matmuls use fp8e4m3 inputs with perf_mode=DoubleRow, which packs
    the K=256 contraction into a single PE pass (2 fp8 weights per cell).
  - The mask outer product is a rank-1 K=1 fp8 matmul (+-224 encodings;
    (-224*224)/256 = -196 -> exp underflows to 0 exactly). K=1 matmuls are
    row-tiled via tile_position so up to 4 run concurrently in the PE array.
  - E is stored centered: e = E - 1 in fp8 (values in [-1, 0.45]), which cuts
    fp8 quantization noise ~12x where it matters. The identity part of
    E = 1 + e is restored algebraically:
        out0^T = q1_8^T @ e  (+ c1A (x) (1-m0) + c1B (x) m0)  [rank-2 f32r MM]
    where c1A = sum_m q1[m,:] (exact f32) and c1B uses the quantized q1 on
    masked columns so the e = -1 cancellation is exact.
  - Out-matmuls run "swapped" (values stationary, e moving) producing out^T
    in PSUM with d on partitions; per-partition c-rows are added by the
    rank-2 matmul, tiles are evicted to bf16, PE-transposed back, and
    normalized by the reciprocal row/col sums (captured for free via the
    exp activation's accum_out).
  - Host-side prep (numpy): fp8 casts, transposed copies, mask/c rows. This
    is layout/sharding work on ~4 MB/core and keeps the device kernel lean.
"""

from contextlib import ExitStack

import numpy as np
import ml_dtypes

import concourse.bass as bass
import concourse.tile as tile
from concourse import bacc, mybir
from concourse.bass_utils import run_bass_kernel_spmd
from concourse.masks import make_identity

P = 128
B = 8
L = 2048          # L0 == L1
D = 256
T = L // P        # 16 row tiles
NCH = 512         # psum bank width in fp32
MC = 224.0        # mask encoding; (-224*224)/256 = -196 -> exp -> exactly 0
SCALE2 = 1.0 / 256.0   # applied to scores inside exp
SCALE1 = 1.0 / 16.0    # applied to the averaged values at the end

f32 = mybir.dt.float32
f32r = mybir.dt.float32r
bf16 = mybir.dt.bfloat16
f8 = mybir.dt.float8e4
MUL = mybir.AluOpType.mult
EXP = mybir.ActivationFunctionType.Exp
DR = mybir.MatmulPerfMode.DoubleRow
DRSW = mybir.MatmulPerfMode.DoubleRowSwInterleave

F8NP = ml_dtypes.float8_e4m3fn


def _emit(tc: tile.TileContext, ctx: ExitStack, io: dict):
    nc = tc.nc

    consts = ctx.enter_context(tc.tile_pool(name="consts", bufs=1))
    qpool = ctx.enter_context(tc.tile_pool(name="qpool", bufs=1))
    epool = ctx.enter_context(tc.tile_pool(name="epool", bufs=1))
    ebf = ctx.enter_context(tc.tile_pool(name="ebf", bufs=4))
    posb_pool = ctx.enter_context(tc.tile_pool(name="posb", bufs=4))
    outsb = ctx.enter_context(tc.tile_pool(name="outsb", bufs=4))

    # ---- input layouts (dram images are already partition-major: straight
    # block copies, chunked so the first score tiles can start early) ----
    q0n = qpool.tile([P, T, D], f8)       # q0 fp8, row l = t*128+p
    q1n = qpool.tile([P, T, D], f8)
    q0t = qpool.tile([P, 2, L], f8)       # q0^T fp8, d = ko*128+ki
    q1t = qpool.tile([P, 2, L], f8)
    # the A-phase rhs (q1t, needed in full) loads first; the lhsT (q0t) is
    # consumed per 128-column tile, so it streams in behind on a third ring
    s0 = io["q0t"].rearrange("p (ko l) -> p ko l", ko=2)
    s1 = io["q1t"].rearrange("p (ko l) -> p ko l", ko=2)
    nc.sync.dma_start(out=q1t[:, 0, :], in_=s1[:, 0, :])
    nc.scalar.dma_start(out=q1t[:, 1, :], in_=s1[:, 1, :])
    nc.sync.dma_start(out=q0t[:, :, 0:256], in_=s0[:, :, 0:256])

    # ---- mask rows (fp8, +-224), replicated at partitions 0/32/64/96 for
    # row-tiled K=1 matmuls; dim1: 0 = -224*m0, 1 = +224*m1 ----
    mtile = consts.tile([128, 2, L], f8)
    for g in range(4):
        nc.sync.dma_start(out=mtile[g * 32 : g * 32 + 1, :, :], in_=io["mrows"][g : g + 1, :, :])
    nc.sync.dma_start(out=q0t[:, :, 256:1024], in_=s0[:, :, 256:1024])
    nc.sync.dma_start(out=q0t[:, :, 1024:2048], in_=s0[:, :, 1024:2048])

    # q natural layouts are first needed by the out1 chains in phase B
    nc.sync.dma_start(out=q0n, in_=io["q0n"].rearrange("p (t d) -> p t d", t=T))
    nc.sync.dma_start(out=q1n, in_=io["q1n"].rearrange("p (t d) -> p t d", t=T))

    # ---- rank-2 correction operands (f32r), pairs at partitions (32g, 32g+1),
    # packed in one dram tensor (8 row DMAs; needed only ~40us in):
    # ctile dim1: 0 = (c1A, c1B) for out0, 1 = (c0A, c0B) for out1
    # rtile dim1: 0 = (1-m0, m0),          1 = (1-m1, m1) ----
    crm = consts.tile([128, 2, D + L], f32r)
    ctile = crm[:, :, 0:D]
    rtile = crm[:, :, D : D + L]
    for g in range(4):
        nc.sync.dma_start(
            out=crm[g * 32 : g * 32 + 2, :, :],
            in_=io["crm"][2 * g : 2 * g + 2, :, :].bitcast(f32r),
        )

    identf = consts.tile([P, P], f32)
    make_identity(nc, identf)
    identb = consts.tile([P, P], bf16)
    nc.vector.tensor_copy(out=identb, in_=identf)

    # ---- e tiles (fp8, centered E-1) and row/col sums ----
    eA = epool.tile([P, T, L], f8)        # [l0-part(t), m]   feeds out1
    eB = epool.tile([P, T, L], f8)        # [l1-part(t), l0]  feeds out0
    denA = consts.tile([P, T], f32)       # rowsum(E) per tile
    denB = consts.tile([P, 2, T], f32)    # per-half partials in phase B
    d16A = consts.tile([P, T], f32)
    d16B = consts.tile([P, T], f32)
    rc0 = consts.tile([P, T], f32)        # (1/16)/rowsum(E)  (out0 denominators)
    rc1 = consts.tile([P, T], f32)        # (1/16)/colsum(E)  (out1 denominators)

    def score_tile_a(s_psum, t):
        ps = s_psum.tile([P, L], f32, tag="sp")
        for c in range(4):
            off = c * NCH
            nc.tensor.matmul(
                ps[:, off : off + NCH],
                lhsT=q0t[:, 0:2, t * P : (t + 1) * P],
                rhs=q1t[:, 0:2, off : off + NCH],
                start=True,
                stop=False,
                perf_mode=DR,
            )
        for c in range(4):
            off = c * NCH
            nc.tensor.matmul(
                ps[:, off : off + NCH],
                lhsT=mtile[c * 32 : c * 32 + 1, 0, t * P : (t + 1) * P],
                rhs=mtile[c * 32 : c * 32 + 1, 1, off : off + NCH],
                start=False,
                stop=True,
                tile_position=(c * 32, 0),
            )
        eb = ebf.tile([P, L], bf16, tag="ebf")
        nc.scalar.activation(
            out=eb, in_=ps, func=EXP, scale=SCALE2, accum_out=denA[:, t : t + 1]
        )
        nc.vector.tensor_scalar_add(out=eA[:, t, :], in0=eb, scalar1=-1.0)

    def score_tile_b(s_psum, t):
        # both half tiles' DR matmuls first, then all four K=1 mask matmuls
        # back-to-back so they row-tile 4x concurrently in the PE array
        pss = []
        for h in range(2):
            ps = s_psum.tile([P, 2 * NCH], f32, tag="sp", name=f"psb{h}")
            pss.append(ps)
            for c in range(2):
                off = h * 1024 + c * NCH
                nc.tensor.matmul(
                    ps[:, c * NCH : (c + 1) * NCH],
                    lhsT=q1t[:, 0:2, t * P : (t + 1) * P],
                    rhs=q0t[:, 0:2, off : off + NCH],
                    start=True,
                    stop=False,
                    perf_mode=DR,
                )
        for h in range(2):
            for c in range(2):
                off = h * 1024 + c * NCH
                g = h * 2 + c
                nc.tensor.matmul(
                    pss[h][:, c * NCH : (c + 1) * NCH],
                    lhsT=mtile[g * 32 : g * 32 + 1, 1, t * P : (t + 1) * P],
                    rhs=mtile[g * 32 : g * 32 + 1, 0, off : off + NCH],
                    start=False,
                    stop=True,
                    tile_position=(g * 32, 0),
                )
        for h in range(2):
            eb = ebf.tile([P, 2 * NCH], bf16, tag="ebf")
            nc.scalar.activation(
                out=eb, in_=pss[h], func=EXP, scale=SCALE2,
                accum_out=denB[:, h, t : t + 1],
            )
            nc.vector.tensor_scalar_add(
                out=eB[:, t, h * 1024 : (h + 1) * 1024], in0=eb, scalar1=-1.0
            )
        if True:
            nc.vector.tensor_tensor(
                out=d16B[:, t : t + 1],
                in0=denB[:, 0, t : t + 1],
                in1=denB[:, 1, t : t + 1],
                op=mybir.AluOpType.add,
            )
            nc.vector.tensor_scalar_mul(
                out=d16B[:, t : t + 1], in0=d16B[:, t : t + 1], scalar1=16.0
            )
            nc.vector.reciprocal(rc1[:, t : t + 1], d16B[:, t : t + 1])

    def out_chain(o_psum, wq, ev, jj, mg, nch, posb):
        po = o_psum.tile([P, NCH], f32, tag="op")
        for k in range(8):
            nc.tensor.matmul(
                po,
                lhsT=wq[:, 2 * k + mg, :],
                rhs=ev[:, 2 * k : 2 * k + 2, nch * NCH : (nch + 1) * NCH],
                start=(k == 0),
                stop=False,
                perf_mode=DRSW,
            )
        nc.tensor.matmul(
            po,
            lhsT=ctile[nch * 32 : nch * 32 + 2, jj, mg * P : (mg + 1) * P],
            rhs=rtile[nch * 32 : nch * 32 + 2, jj, nch * NCH : (nch + 1) * NCH],
            start=False,
            stop=True,
            tile_position=(nch * 32, 0),
        )
        nc.scalar.copy(out=posb[mg][:, nch * NCH : (nch + 1) * NCH], in_=po)

    def finish_pair(t_psum, posb, rc, odram, t, split_store=False):
        # two adjacent row tiles: 4 transposes -> one psum buf -> 2 scaled
        # copies -> a single batched store
        tp = t_psum.tile([P, 2, D], bf16, tag="tp")
        for u in range(2):
            for mg in range(2):
                nc.tensor.transpose(
                    tp[:, u, mg * P : (mg + 1) * P],
                    posb[mg][:, (t + u) * P : (t + u + 1) * P],
                    identb,
                )
        ob = outsb.tile([P, 2, D], f32, tag="ob")
        for u in range(2):
            nc.scalar.activation(
                out=ob[:, u, :], in_=tp[:, u, :],
                func=mybir.ActivationFunctionType.Copy,
                scale=rc[:, t + u : t + u + 1],
            )
        if split_store:
            nc.sync.dma_start(out=odram[t * P : (t + 1) * P, :], in_=ob[:, 0, :])
            nc.scalar.dma_start(out=odram[(t + 1) * P : (t + 2) * P, :], in_=ob[:, 1, :])
        else:
            nc.sync.dma_start(
                out=odram[t * P : (t + 2) * P, :].rearrange("(u p) d -> p u d", p=P),
                in_=ob,
            )

    posb1 = [
        posb_pool.tile([P, L], bf16, tag=f"po1{mg}", name=f"posb1_{mg}")
        for mg in range(2)
    ]
    posb0 = [
        posb_pool.tile([P, L], bf16, tag=f"po0{mg}", name=f"posb0_{mg}")
        for mg in range(2)
    ]

    # ---- phase A: orientation A scores (S[l0, m] -> eA, rowsum -> rc0) ----
    with tc.tile_pool(name="s_psumA", bufs=2, space="PSUM") as s_psumA:
        for t in range(T):
            score_tile_a(s_psumA, t)
    # rc0 = 1/(16*rowsum(E)), batched (rc0 is only read by the out0 finishes)
    nc.vector.tensor_scalar_mul(out=d16A, in0=denA, scalar1=16.0)
    nc.vector.reciprocal(rc0, d16A)

    # ---- phase B: orientation B scores interleaved with out1 work (out1
    # depends only on eA, so its matmuls fill the PE behind the B exps).
    # out1's 8 accumulation chains (9 MMs each) are dribbled out ~5 MMs per
    # score tile; once both mg-chunks of a 512-column group are evicted, its
    # finish tiles (transpose + normalize + store) ride along as well. ----
    o_psum = ctx.enter_context(tc.tile_pool(name="o_psum", bufs=2, space="PSUM"))
    t_psum = ctx.enter_context(tc.tile_pool(name="t_psum", bufs=2, space="PSUM"))
    with tc.tile_pool(name="s_psumB", bufs=2, space="PSUM") as s_psumB:
        mm_cursor = [0]  # flat index over out1's 8 chains x 9 matmuls
        po_cur = [None]
        rc1_ready = [-1]          # highest tile whose rc1 has been emitted
        pending_fin = []          # nch groups whose finishes await rc1

        def flush_finishes():
            rest = []
            for nch in pending_fin:
                if 4 * nch + 3 <= rc1_ready[0]:
                    for tt in range(nch * 4, nch * 4 + 4, 2):
                        finish_pair(t_psum, posb1, rc1, io["out1"], tt)
                else:
                    rest.append(nch)
            pending_fin[:] = rest

        def emit_out1_mms(n):
            for _ in range(n):
                idx = mm_cursor[0]
                if idx >= 72:
                    return
                j, k = divmod(idx, 9)
                mg, nch = j % 2, j // 2
                if k == 0:
                    po_cur[0] = o_psum.tile([P, NCH], f32, tag="op", name="po1c")
                po = po_cur[0]
                if k < 8:
                    nc.tensor.matmul(
                        po,
                        lhsT=q0n[:, 2 * k + mg, :],
                        rhs=eA[:, 2 * k : 2 * k + 2, nch * NCH : (nch + 1) * NCH],
                        start=(k == 0),
                        stop=False,
                        perf_mode=DRSW,
                    )
                else:
                    nc.tensor.matmul(
                        po,
                        lhsT=ctile[nch * 32 : nch * 32 + 2, 1, mg * P : (mg + 1) * P],
                        rhs=rtile[nch * 32 : nch * 32 + 2, 1, nch * NCH : (nch + 1) * NCH],
                        start=False,
                        stop=True,
                        tile_position=(nch * 32, 0),
                    )
                    nc.scalar.copy(
                        out=posb1[mg][:, nch * NCH : (nch + 1) * NCH], in_=po
                    )
                    if mg == 1:
                        pending_fin.append(nch)
                        flush_finishes()
                mm_cursor[0] += 1

        for t in range(T):
            score_tile_b(s_psumB, t)
            rc1_ready[0] = t
            if t >= 1:
                emit_out1_mms(5 if t < 15 else 72)
                flush_finishes()
        emit_out1_mms(72)
        flush_finishes()
        assert not pending_fin and mm_cursor[0] >= 72

        # ---- out0 chains with finish pairs staggered one pair behind ----
        next_pair = [0]

        def flush_out0_pairs(upto):
            while next_pair[0] <= upto:
                finish_pair(
                    t_psum, posb0, rc0, io["out0"], next_pair[0] * 2,
                    split_store=(next_pair[0] >= 6),
                )
                next_pair[0] += 1

        for nch in range(4):
            for mg in range(2):
                out_chain(o_psum, q1n, eB, 0, mg, nch, posb0)
            flush_out0_pairs(2 * nch - 1)  # lag one group behind ready
        flush_out0_pairs(7)


_CACHED_NC = None


def _build():
    global _CACHED_NC
    if _CACHED_NC is not None:
        return _CACHED_NC
    nc = bacc.Bacc("TRN2", target_bir_lowering=False, debug=False)
    io = {
        "q0n": nc.dram_tensor("q0n", [P, T * D], f8, kind="ExternalInput").ap(),
        "q1n": nc.dram_tensor("q1n", [P, T * D], f8, kind="ExternalInput").ap(),
        "q0t": nc.dram_tensor("q0t", [P, 2 * L], f8, kind="ExternalInput").ap(),
        "q1t": nc.dram_tensor("q1t", [P, 2 * L], f8, kind="ExternalInput").ap(),
        "mrows": nc.dram_tensor("mrows", [4, 2, L], f8, kind="ExternalInput").ap(),
        "crm": nc.dram_tensor("crm", [8, 2, D + L], f32, kind="ExternalInput").ap(),
        "out0": nc.dram_tensor("out0", [L, D], f32, kind="ExternalOutput").ap(),
        "out1": nc.dram_tensor("out1", [L, D], f32, kind="ExternalOutput").ap(),
    }
    with tile.TileContext(nc) as tc:
        with ExitStack() as ctx:
            _emit(tc, ctx, io)
    nc.compile()
    _CACHED_NC = nc
    return nc


def _prep_inputs(q0, q1, m0, m1):
    """Host-side sharding/layout prep for one batch element (numpy)."""
    q0_8 = q0.astype(F8NP)
    q1_8 = q1.astype(F8NP)
    q0_8f = q0_8.astype(np.float32)
    q1_8f = q1_8.astype(np.float32)
    m0f = m0.astype(np.float32)
    m1f = m1.astype(np.float32)

    mrows = np.empty([4, 2, L], F8NP)
    mrows[:, 0, :] = (-MC * m0f).astype(F8NP)[None, :]
    mrows[:, 1, :] = (MC * m1f).astype(F8NP)[None, :]

    # c-rows: identity-part restoration. A-row: all columns at full precision;
    # B-row: masked columns use the quantized values so the e=-1 cancellation
    # in masked rows is exact.
    c1A = q1.sum(0)
    c1B = np.where(m1f[:, None] == 1.0, q1_8f, q1).sum(0)
    c0A = q0.sum(0)
    c0B = np.where(m0f[:, None] == 1.0, q0_8f, q0).sum(0)
    crm = np.empty([8, 2, D + L], np.float32)
    for g in range(4):
        crm[2 * g, 0, :D] = c1A
        crm[2 * g + 1, 0, :D] = c1B
        crm[2 * g, 1, :D] = c0A
        crm[2 * g + 1, 1, :D] = c0B
        crm[2 * g, 0, D:] = 1.0 - m0f
        crm[2 * g + 1, 0, D:] = m0f
        crm[2 * g, 1, D:] = 1.0 - m1f
        crm[2 * g + 1, 1, D:] = m1f

    # partition-major dram images: [p, (t, d)] for the natural layout and
    # [ki, (ko, l)] for the transposed layout -> straight [128, X] DMAs
    def nat_img(q8a):
        # sw-interleaved out-chain weight windows: j = 2k+mg -> [128, 256]
        # (pairs from m-tiles 2k/2k+1, d-columns mg*128..+128, interleaved
        # with columns reversed as the DoubleRowSwInterleave path expects)
        qn = q8a.reshape(T, P, D).transpose(1, 0, 2)  # [ki, t, d]
        win = np.empty([P, T, 2 * P], q8a.dtype)
        for k in range(8):
            for mg in range(2):
                a = qn[:, 2 * k, mg * P : (mg + 1) * P]
                b = qn[:, 2 * k + 1, mg * P : (mg + 1) * P]
                win[:, 2 * k + mg, 0::2] = a[:, ::-1]
                win[:, 2 * k + mg, 1::2] = b[:, ::-1]
        return np.ascontiguousarray(win.reshape(P, T * D))

    def t_img(q8a):
        return np.ascontiguousarray(
            q8a.T.reshape(2, P, L).transpose(1, 0, 2).reshape(P, 2 * L)
        )

    return {
        "q0n": nat_img(q0_8),
        "q1n": nat_img(q1_8),
        "q0t": t_img(q0_8),
        "q1t": t_img(q1_8),
        "mrows": mrows,
        "crm": crm,
    }


def run_on_cores(q0, q1, mask0, mask1, trace=False):
    """Run the SPMD kernel; returns (out0, out1, BassKernelResults)."""
    nc = _build()
    in_maps = [
        _prep_inputs(
            np.asarray(q0[b], dtype=np.float32),
            np.asarray(q1[b], dtype=np.float32),
            np.asarray(mask0[b], dtype=np.int32),
            np.asarray(mask1[b], dtype=np.int32),
        )
        for b in range(B)
    ]
    br = run_bass_kernel_spmd(nc, in_maps, list(range(B)), trace=trace)
    out0 = np.stack([br.results[b]["out0"] for b in range(B)])
    out1 = np.stack([br.results[b]["out1"] for b in range(B)])
    return out0, out1, br


def kernel(q0, q1, len0=None, len1=None, mask0=None, mask1=None, **_):
    q0 = np.asarray(q0, dtype=np.float32)
    q1 = np.asarray(q1, dtype=np.float32)
    mask0 = np.asarray(mask0, dtype=np.int32)
    mask1 = np.asarray(mask1, dtype=np.int32)
    out0, out1, _br = run_on_cores(q0, q1, mask0, mask1, trace=False)
    return out0, out1
